# revision 19
# baseline (speedup 1.0000x reference)
"""Trainium2 Bass kernel for the DifferentiableTVLayer PDHG solve.

Accelerated Chambolle-Pock (strong-convexity schedule, gamma=0.3) brings the
iteration count from 200 to N_ITERS while staying inside the 2e-2 relative
error budget. The primal state is kept pre-scaled (U~ = u / s_n with
s_{n+1} = s_n/(1+tau_n)) so the contraction and the tau_n*f term disappear
from the matmul path: the difference-operator matrices are constant +-1,
schedule dependence rides on STT immediates and on per-iteration scaled
identity weight tiles (A_n*I / -B_n*I) for the extrapolation.

Per-core layout ("layout A"): SBUF tiles [128, 512] where
    tile[p, c*256 + h] = X[h, w]  with  w = c*128 + p,  c in {0,1}.
The image is processed as two W-chunk halves (free cols [0:256) / [256:512)).

Per iteration n (per half x):
    PSq_x = I@Q_x + Ly@VB_x (+ Ey@VB_B for x=0)            (PE, fp16)
    Q'_x  = clip(PSq_x, +-LY)                              (DVE fused clip)
    P'_x  = clip(P + dxF(VB), +-LX)                        (DVE: sub, add, clip)
    PSu_x = (-f preload, ACT) + I@Ppad - I@Ppad' + MyT@Q' (+EyT@Q'_A) = D - f
    U~'   = (PSu * c_n) + U~                               (DVE STT)
    PSv_x = (A_n I)@U~' + (-B_n I)@U~                      (PE, per-iter weights)
    VB'   = copy(PSv_x) -> fp16                            (ACT)
"""

import numpy as np

import concourse.bass as bass
import concourse.mybir as mybir
from concourse.tile import TileContext

B, H, W = 8, 256, 256
P, NCH = 128, 2
FREE = NCH * H  # 512

N_ITERS = 44
TAU0 = 0.35355339
SIGMA0 = 1.0 / (8.0 * TAU0)
GAMMA = 0.3

F32 = mybir.dt.float32
F32R = mybir.dt.float32r
F16 = mybir.dt.float16
AOP = mybir.AluOpType

USE_FUSED_CLIP = False
PRELOAD = "act"   # "act" | "mm"


def _schedule(n_iters=N_ITERS):
    """Per-iteration scalars for the UU-state (raw PSu copy) formulation.

    UU(n) = U~_n / c_n lives in SBUF; PSu(n) = UU(n) + D - f (U injected by
    a constant identity matmul); U~_{n+1} = c_n PSu(n).
      VB'   = A c_n PSu - B c_n UU = g2*((PSu * g1) + UU),
              g1 = -A/B, g2 = -B*c_n
      UU'   = (c_n / c_{n+1}) * PSu            (tensor_scalar)
    Returns per-iter (g1, g2, w_next) plus (u0_scale, out_scale).
    """
    tau, sigma, s = TAU0, SIGMA0, 1.0
    cs, ABs = [], []
    for _ in range(n_iters):
        theta = 1.0 / np.sqrt(1.0 + 2.0 * GAMMA * tau)
        s_next = s / (1.0 + tau)
        sig_next = sigma / theta
        cs.append(-tau / s)
        A_n = sig_next * (1.0 + theta) * s_next
        B_n = sig_next * theta * s
        ABs.append((A_n, B_n))
        tau, sigma, s = tau * theta, sig_next, s_next
    s_final = s
    out = []
    for i in range(n_iters):
        A_n, B_n = ABs[i]
        g1 = -A_n / B_n
        g2 = -B_n * cs[i]
        w = cs[i] / cs[i + 1] if i + 1 < n_iters else 0.0
        out.append((float(g1), float(g2), float(w)))
    u0_scale = 1.0 / cs[0]
    out_scale = s_final * cs[-1]
    return out, (float(u0_scale), float(out_scale))


SCHED, SCALES = _schedule()


# ---------------------------------------------------------------- host layout
def _to_layout_a(x):
    """[H, W] -> [128, 512]: out[p, c*256+h] = x[h, c*128+p]."""
    return np.ascontiguousarray(
        x.T.reshape(NCH, P, H).transpose(1, 0, 2).reshape(P, FREE)
    )


def _from_layout_a(t):
    return np.ascontiguousarray(
        t.reshape(P, NCH, H).transpose(1, 0, 2).reshape(W, H).T
    )


def _make_matrices():
    """Constant lhsT operators [k, m]: out[m] = sum_k lhsT[k,m] rhs[k].
    Packed side by side into one [128, 5*128] fp16 tensor."""
    Ly = np.zeros((P, P), np.float16)
    for m in range(P):
        Ly[m, m] = -1.0
        if m + 1 < P:
            Ly[m + 1, m] = 1.0
    Ey = np.zeros((P, P), np.float16)
    Ey[0, 127] = 1.0
    MyT = np.zeros((P, P), np.float16)   # PS accumulates +dyT q
    for m in range(P):
        MyT[m, m] = -1.0
        if m - 1 >= 0:
            MyT[m - 1, m] = 1.0
    EyT = np.zeros((P, P), np.float16)
    EyT[127, 0] = 1.0
    I = np.eye(P, dtype=np.float16)
    NI = (-I).astype(np.float16)
    packed = np.concatenate([I, NI, Ly, Ey, MyT, EyT], axis=1)
    order = {"mI": 0, "mNI": 1, "mLy": 2, "mEy": 3, "mMyT": 4, "mEyT": 5}
    return packed, order, np.eye(P, dtype=np.float32)


def _make_ab_weights(sched):
    """Per-iteration (A_n*I | -B_n*I) fp16 weight blocks, packed."""
    n = len(sched)
    out = np.zeros((P, 2 * n * P), np.float32)
    for i, (_, A_n, B_n) in enumerate(sched):
        out[:, (2 * i) * P:(2 * i + 1) * P] = A_n * np.eye(P)
        out[:, (2 * i + 1) * P:(2 * i + 2) * P] = -B_n * np.eye(P)
    return out


def _per_core_inputs(f_img, lam_img):
    fa = _to_layout_a(f_img).astype(np.float32)

    lamx = np.concatenate([lam_img[1:, :], np.zeros((1, W), np.float32)])
    lx3 = _to_layout_a(lamx).reshape(P, NCH, H).copy()
    lx3[:, :, 255] = 0.0
    lx = lx3.reshape(P, FREE).astype(np.float16)

    lamy = np.concatenate([lam_img[:, 1:], np.zeros((H, 1), np.float32)], axis=1)
    ly = _to_layout_a(lamy).astype(np.float16)  # (c=1, p=127) col already zero

    u0s, _ = SCALES
    return {
        "u0": np.ascontiguousarray(u0s * fa),
        "nf": np.ascontiguousarray(-fa),
        "vb0": (SIGMA0 * fa).astype(np.float16),
        "lx": np.ascontiguousarray(lx),
        "ly": np.ascontiguousarray(ly),
    }


# ---------------------------------------------------------------- custom op
def _register_clip_op():
    """out = clip(in0, -in1, +in1) as a single DVE instruction."""
    from concourse import dve_ops
    from concourse.dve_spec import Spec, Src0, Src1, maxx, minn, lower
    from concourse.dve_uop import DveOpSpec

    for op in dve_ops.OPS:
        if op.name == "TV_CLIP2_ANT":
            return op
    spec = Spec(
        body=minn(maxx(Src0, -Src1), Src1),
        reference=lambda in0, in1, s0, s1, imm2: np.minimum(
            np.maximum(in0, -in1), in1
        ).astype(np.float32),
    )
    op = dve_ops.DveOp("TV_CLIP2_ANT", spec, subdim=False, uops_sha={})
    dve_ops.OPS.append(op)
    dve_ops.CUSTOM_DVE_SPECS[op.name] = spec
    dve_ops._SUB_OPCODE_FOR_NAME[op.name] = (
        max(dve_ops._SUB_OPCODE_FOR_NAME.values()) + 1
    )
    for ver in ("v3", "v4"):
        try:
            s = DveOpSpec(
                name=op.name,
                opcode=dve_ops.get_dve_sub_opcode(op.name),
                uops=lower(spec, ver=ver),
                rd1_en=True,
            )
            op.uops_sha[ver] = s.sha(ver)
        except Exception:
            pass
    return op


try:
    CLIP_OP = _register_clip_op()
except Exception:
    CLIP_OP = None


# ---------------------------------------------------------------- bass build
def split_excess_waits(nc, max_waits=1):
    """This neuronxcc/walrus build encodes at most ONE sync wait per
    instruction; split the excess onto NoOp carriers."""
    nsplit = 0
    for f in nc.m.functions:
        for bb in f.blocks:
            il = bb.instructions
            out = []
            for inst in il:
                si = inst.sync_info
                waits = list(si.on_wait) if si and si.on_wait else []
                k = 0
                while len(waits) > max_waits:
                    head, waits = waits[:max_waits], waits[max_waits:]
                    out.append(
                        mybir.InstNoOp(
                            name=f"{inst.name}-waitsplit{k}",
                            engine=inst.engine,
                            ins=[],
                            outs=[],
                            sync_info=mybir.SyncInfo(on_wait=head, on_update=[]),
                        )
                    )
                    k += 1
                    nsplit += 1
                if k:
                    inst.sync_info = mybir.SyncInfo(
                        on_wait=waits,
                        on_update=list(si.on_update) if si.on_update else [],
                    )
                out.append(inst)
            il[:] = out
    return nsplit


def build_nc(n_iters=N_ITERS, split=True):
    sched, (u0_scale, out_scale) = _schedule(n_iters)
    nc = bass.Bass(trn_type="TRN2")

    d_in32 = {
        name: nc.dram_tensor(name, [P, FREE], F32, kind="ExternalInput")
        for name in ("u0", "nf")
    }
    d_in16 = {
        name: nc.dram_tensor(name, [P, FREE], F16, kind="ExternalInput")
        for name in ("vb0", "lx", "ly")
    }
    d_out = nc.dram_tensor("out", [P, FREE], F32, kind="ExternalOutput")
    mat_packed, mat_order, i32 = _make_matrices()
    d_mats = nc.inline_tensor(mat_packed, name="mats")
    d_i32 = nc.inline_tensor(i32, name="i32")

    with TileContext(nc) as tc:
        with (
            tc.tile_pool(name="state", bufs=1) as state,
            tc.tile_pool(name="scratch", bufs=4) as scratch,
            tc.tile_pool(name="psA", bufs=2, space="PSUM") as psA,
            tc.tile_pool(name="psB", bufs=2, space="PSUM") as psB,
        ):
            UUs = [state.tile([P, FREE], F32R, name=f"UU{i}")
                   for i in range(2)]
            MI32 = state.tile([P, P], F32R, name="MI32")
            VBs = [state.tile([P, FREE], F16, name=f"VB{i}") for i in range(2)]
            Ps = [state.tile([P, FREE + 4], F16, name=f"Pd{i}") for i in range(2)]
            Qs = [state.tile([P, FREE], F16, name=f"Qd{i}") for i in range(2)]
            LX = state.tile([P, FREE], F16, name="LX")
            LY = state.tile([P, FREE], F16, name="LY")
            NLX = state.tile([P, FREE], F16, name="NLX")
            NLY = state.tile([P, FREE], F16, name="NLY")
            NF = state.tile([P, FREE], F32, name="NF")
            MATS = state.tile([P, 6 * P], F16, name="MATS")

            def MAT(name):
                k = mat_order[name]
                return MATS[:, k * P:(k + 1) * P]

            # ---- setup
            nc.gpsimd.dma_start(out=UUs[0], in_=d_in32["u0"].ap())
            nc.gpsimd.dma_start(out=MI32, in_=d_i32.ap())
            nc.sync.dma_start(out=NF, in_=d_in32["nf"].ap())
            nc.gpsimd.dma_start(out=VBs[0], in_=d_in16["vb0"].ap())
            nc.gpsimd.dma_start(out=LX, in_=d_in16["lx"].ap())
            nc.gpsimd.dma_start(out=LY, in_=d_in16["ly"].ap())
            nc.sync.dma_start(out=MATS, in_=d_mats.ap())
            nc.scalar.mul(NLX, LX, -1.0)
            nc.scalar.mul(NLY, LY, -1.0)
            nc.vector.memset(Ps[0].bitcast(F32), 0.0)
            nc.vector.memset(Ps[1].bitcast(F32), 0.0)
            nc.vector.memset(Qs[0].bitcast(F32), 0.0)

            OutT = state.tile([P, FREE], F32, name="OutT")
            OUT_HALVES = [OutT[:, 0:H], OutT[:, H:FREE]]

            def mm(out, lhsT, rhs, start, stop):
                nc.tensor.matmul(
                    out, lhsT, rhs, start=start, stop=stop,
                    skip_group_check=True,
                )

            def clip(out, val, lam, nlam, mintag):
                if USE_FUSED_CLIP and CLIP_OP is not None:
                    nc.vector._custom_dve(CLIP_OP, out=out, in0=val, in1=lam)
                else:
                    Mn = scratch.tile([P, H], F16, name=mintag, tag=mintag)
                    n = val.shape[-1]
                    nc.vector.tensor_tensor(Mn[:, 0:n], val, lam, AOP.min)
                    nc.vector.tensor_tensor(out, Mn[:, 0:n], nlam, AOP.max)

            for i in range(n_iters):
                g1_n, g2_n, w_n = sched[i]
                a, b = i % 2, (i + 1) % 2
                UUc, UUn = UUs[a], UUs[b]
                VBc, VBn = VBs[a], VBs[b]
                Pc, Pn = Ps[a], Ps[b]
                Qc, Qn = Qs[a], Qs[b]

                PSq = [None, None]
                for x in (0, 1):
                    lo, hi = x * H, (x + 1) * H
                    # ---- dual q (partition-dim, PE)
                    PSq[x] = psA.tile([P, H], F32, name=f"PSq{x}",
                                      tag=f"PSq{x}")
                    mm(PSq[x], MAT("mI"), Qc[:, lo:hi], start=True, stop=False)
                    mm(PSq[x], MAT("mLy"), VBc[:, lo:hi], start=False,
                       stop=(x == 1))
                    if x == 0:
                        mm(PSq[x], MAT("mEy"), VBc[:, H:FREE], start=False,
                           stop=True)

                    # ---- dual p (free-dim shifts, DVE fp16 2x)
                    G = scratch.tile([P, H], F16, name=f"G{x}", tag=f"G{x}")
                    nc.vector.tensor_sub(
                        G[:, 0:255], VBc[:, lo + 1:hi], VBc[:, lo:hi - 1]
                    )
                    Ppre = scratch.tile([P, H], F16, name=f"Pp{x}",
                                        tag=f"Pp{x}")
                    nc.vector.tensor_add(
                        Ppre[:, 0:255], G[:, 0:255], Pc[:, 1 + lo:hi]
                    )
                    clip(Pn[:, 1 + lo:hi], Ppre[:, 0:255], LX[:, lo:hi - 1],
                         NLX[:, lo:hi - 1], f"Pm{x}")
                    # per-chunk col 255 of Pn stays 0 (zeroed at setup).

                for x in (0, 1):
                    lo, hi = x * H, (x + 1) * H
                    # ---- q clip (reads PSUM)
                    clip(Qn[:, lo:hi], PSq[x], LY[:, lo:hi], NLY[:, lo:hi],
                         f"Qm{x}")

                for x in (0, 1):
                    lo, hi = x * H, (x + 1) * H
                    # ---- primal accumulation: PS = D - f
                    PSu = psB.tile([P, H], F32, name=f"PSu{x}", tag=f"PSu{x}")
                    if PRELOAD == "act":
                        nc.scalar.copy(PSu, NF[:, lo:hi])
                        first = False
                    else:
                        mm(PSu, MAT("mI"), NF[:, lo:hi], start=True,
                           stop=False)
                        first = False
                    mm(PSu, MI32, UUc[:, lo:hi], start=False, stop=False)
                    mm(PSu, MAT("mI"), Pn[:, lo:hi], start=False, stop=False)
                    mm(PSu, MAT("mNI"), Pn[:, 1 + lo:1 + hi], start=False,
                       stop=False)
                    mm(PSu, MAT("mMyT"), Qn[:, lo:hi], start=False,
                       stop=(x == 0))
                    if x == 1:
                        mm(PSu, MAT("mEyT"), Qn[:, 0:H], start=False,
                           stop=True)

                    if i + 1 < n_iters:
                        # ---- extrapolation first (gates the next iteration)
                        Tv = scratch.tile([P, H], F32, name=f"Tv{x}",
                                          tag=f"Tv{x}")
                        nc.vector.scalar_tensor_tensor(
                            out=Tv, in0=PSu, scalar=g1_n, in1=UUc[:, lo:hi],
                            op0=AOP.mult, op1=AOP.add,
                        )
                        nc.scalar.mul(VBn[:, lo:hi], Tv, float(g2_n))
                        # ---- primal state: UU' = w_n * PSu (slack)
                        nc.vector.tensor_scalar_mul(
                            UUn[:, lo:hi], PSu, float(w_n)
                        )
                    else:
                        nc.scalar.mul(OUT_HALVES[x], PSu, float(out_scale))

            nc.sync.dma_start(out=d_out.ap(), in_=OutT)

    nc.finalize()
    if split:
        split_excess_waits(nc)
    return nc


_NC_CACHE = {}


def _get_nc(n_iters=N_ITERS):
    key = n_iters
    if key not in _NC_CACHE:
        _NC_CACHE[key] = build_nc(n_iters)
    return _NC_CACHE[key]


def kernel(f, lam):
    from concourse.bass_utils import run_bass_kernel_spmd

    f = np.asarray(f, dtype=np.float32)
    lam = np.asarray(lam, dtype=np.float32)
    nc = _get_nc()
    in_maps = [_per_core_inputs(f[b], lam[b]) for b in range(B)]
    res = run_bass_kernel_spmd(nc, in_maps, core_ids=list(range(B)))
    return np.stack([_from_layout_a(res.results[b]["out"]) for b in range(B)])


if __name__ == "__main__":
    import sys

    if "--build" in sys.argv:
        import time

        t0 = time.time()
        nc = build_nc(int(sys.argv[sys.argv.index("--build") + 1])
                      if len(sys.argv) > 2 else N_ITERS)
        print(f"build ok in {time.time()-t0:.1f}s")


# revision 20
# speedup vs baseline: 1.2904x; 1.2904x over previous
"""Trainium2 Bass kernel for the DifferentiableTVLayer PDHG solve.

Accelerated Chambolle-Pock (strong-convexity schedule, gamma=0.3) brings the
iteration count from 200 to N_ITERS while staying inside the 2e-2 relative
error budget. The primal state is kept pre-scaled (U~ = u / s_n with
s_{n+1} = s_n/(1+tau_n)) so the contraction and the tau_n*f term disappear
from the matmul path: the difference-operator matrices are constant +-1,
schedule dependence rides on STT immediates and on per-iteration scaled
identity weight tiles (A_n*I / -B_n*I) for the extrapolation.

Per-core layout ("layout A"): SBUF tiles [128, 512] where
    tile[p, c*256 + h] = X[h, w]  with  w = c*128 + p,  c in {0,1}.
The image is processed as two W-chunk halves (free cols [0:256) / [256:512)).

Per iteration n (per half x):
    PSq_x = I@Q_x + Ly@VB_x (+ Ey@VB_B for x=0)            (PE, fp16)
    Q'_x  = clip(PSq_x, +-LY)                              (DVE fused clip)
    P'_x  = clip(P + dxF(VB), +-LX)                        (DVE: sub, add, clip)
    PSu_x = (-f preload, ACT) + I@Ppad - I@Ppad' + MyT@Q' (+EyT@Q'_A) = D - f
    U~'   = (PSu * c_n) + U~                               (DVE STT)
    PSv_x = (A_n I)@U~' + (-B_n I)@U~                      (PE, per-iter weights)
    VB'   = copy(PSv_x) -> fp16                            (ACT)
"""

import numpy as np

import concourse.bass as bass
import concourse.mybir as mybir
from concourse.tile import TileContext

B, H, W = 8, 256, 256
P, NCH = 128, 2
FREE = NCH * H  # 512

N_ITERS = 44
TAU0 = 0.35355339
SIGMA0 = 1.0 / (8.0 * TAU0)
GAMMA = 0.3

F32 = mybir.dt.float32
F32R = mybir.dt.float32r
F16 = mybir.dt.float16
AOP = mybir.AluOpType

USE_FUSED_CLIP = False
PRELOAD = "act"   # "act" | "mm"


def _schedule(n_iters=N_ITERS):
    """Per-iteration scalars for the UU-state (raw PSu copy) formulation.

    UU(n) = U~_n / c_n lives in SBUF; PSu(n) = UU(n) + D - f (U injected by
    a constant identity matmul); U~_{n+1} = c_n PSu(n).
      VB'   = A c_n PSu - B c_n UU = (PSu * m1) + UU2   (DVE STT, fp16 out)
      UU'   = w  * PSu   (ACT scale-copy)
      UU2'  = w2 * PSu   (ACT scale-copy; UU2 = g2*UU pre-scaled)
    Returns per-iter (m1, w, w2) plus (u0_scale, out_scale, g2_0).
    """
    tau, sigma, s = TAU0, SIGMA0, 1.0
    cs, ABs = [], []
    for _ in range(n_iters):
        theta = 1.0 / np.sqrt(1.0 + 2.0 * GAMMA * tau)
        s_next = s / (1.0 + tau)
        sig_next = sigma / theta
        cs.append(-tau / s)
        A_n = sig_next * (1.0 + theta) * s_next
        B_n = sig_next * theta * s
        ABs.append((A_n, B_n))
        tau, sigma, s = tau * theta, sig_next, s_next
    s_final = s
    g2s = [-ABs[i][1] * cs[i] for i in range(n_iters)]
    out = []
    for i in range(n_iters):
        A_n, B_n = ABs[i]
        m1 = A_n * cs[i]                      # VB' = m1*PSu + UU2
        w = cs[i] / cs[i + 1] if i + 1 < n_iters else 0.0
        w2 = g2s[i + 1] * w if i + 1 < n_iters else 0.0
        out.append((float(m1), float(w), float(w2)))
    u0_scale = 1.0 / cs[0]
    out_scale = s_final * cs[-1]
    return out, (float(u0_scale), float(out_scale), float(g2s[0]))


SCHED, SCALES = _schedule()


# ---------------------------------------------------------------- host layout
def _to_layout_a(x):
    """[H, W] -> [128, 512]: out[p, c*256+h] = x[h, c*128+p]."""
    return np.ascontiguousarray(
        x.T.reshape(NCH, P, H).transpose(1, 0, 2).reshape(P, FREE)
    )


def _from_layout_a(t):
    return np.ascontiguousarray(
        t.reshape(P, NCH, H).transpose(1, 0, 2).reshape(W, H).T
    )


def _make_matrices():
    """Constant lhsT operators [k, m]: out[m] = sum_k lhsT[k,m] rhs[k].
    Packed side by side into one [128, 5*128] fp16 tensor."""
    Ly = np.zeros((P, P), np.float16)
    for m in range(P):
        Ly[m, m] = -1.0
        if m + 1 < P:
            Ly[m + 1, m] = 1.0
    Ey = np.zeros((P, P), np.float16)
    Ey[0, 127] = 1.0
    MyT = np.zeros((P, P), np.float16)   # PS accumulates +dyT q
    for m in range(P):
        MyT[m, m] = -1.0
        if m - 1 >= 0:
            MyT[m - 1, m] = 1.0
    EyT = np.zeros((P, P), np.float16)
    EyT[127, 0] = 1.0
    I = np.eye(P, dtype=np.float16)
    NI = (-I).astype(np.float16)
    packed = np.concatenate([I, NI, Ly, Ey, MyT, EyT], axis=1)
    order = {"mI": 0, "mNI": 1, "mLy": 2, "mEy": 3, "mMyT": 4, "mEyT": 5}
    return packed, order, np.eye(P, dtype=np.float32)


def _make_ab_weights(sched):
    """Per-iteration (A_n*I | -B_n*I) fp16 weight blocks, packed."""
    n = len(sched)
    out = np.zeros((P, 2 * n * P), np.float32)
    for i, (_, A_n, B_n) in enumerate(sched):
        out[:, (2 * i) * P:(2 * i + 1) * P] = A_n * np.eye(P)
        out[:, (2 * i + 1) * P:(2 * i + 2) * P] = -B_n * np.eye(P)
    return out


def _per_core_inputs(f_img, lam_img):
    fa = _to_layout_a(f_img).astype(np.float32)

    lamx = np.concatenate([lam_img[1:, :], np.zeros((1, W), np.float32)])
    lx3 = _to_layout_a(lamx).reshape(P, NCH, H).copy()
    lx3[:, :, 255] = 0.0
    lx = lx3.reshape(P, FREE).astype(np.float16)

    lamy = np.concatenate([lam_img[:, 1:], np.zeros((H, 1), np.float32)], axis=1)
    ly = _to_layout_a(lamy).astype(np.float16)  # (c=1, p=127) col already zero

    u0s = SCALES[0]
    return {
        "u0": np.ascontiguousarray(u0s * fa),
        "nf": np.ascontiguousarray(-fa),
        "vb0": (SIGMA0 * fa).astype(np.float16),
        "lx": np.ascontiguousarray(lx),
        "ly": np.ascontiguousarray(ly),
    }


# ---------------------------------------------------------------- custom op
def _register_clip_op():
    """out = clip(in0, -in1, +in1) as a single DVE instruction."""
    from concourse import dve_ops
    from concourse.dve_spec import Spec, Src0, Src1, maxx, minn, lower
    from concourse.dve_uop import DveOpSpec

    for op in dve_ops.OPS:
        if op.name == "TV_CLIP2_ANT":
            return op
    spec = Spec(
        body=minn(maxx(Src0, -Src1), Src1),
        reference=lambda in0, in1, s0, s1, imm2: np.minimum(
            np.maximum(in0, -in1), in1
        ).astype(np.float32),
    )
    op = dve_ops.DveOp("TV_CLIP2_ANT", spec, subdim=False, uops_sha={})
    dve_ops.OPS.append(op)
    dve_ops.CUSTOM_DVE_SPECS[op.name] = spec
    dve_ops._SUB_OPCODE_FOR_NAME[op.name] = (
        max(dve_ops._SUB_OPCODE_FOR_NAME.values()) + 1
    )
    for ver in ("v3", "v4"):
        try:
            s = DveOpSpec(
                name=op.name,
                opcode=dve_ops.get_dve_sub_opcode(op.name),
                uops=lower(spec, ver=ver),
                rd1_en=True,
            )
            op.uops_sha[ver] = s.sha(ver)
        except Exception:
            pass
    return op


try:
    CLIP_OP = _register_clip_op()
except Exception:
    CLIP_OP = None


# ---------------------------------------------------------------- bass build
def split_excess_waits(nc, max_waits=1):
    """This neuronxcc/walrus build encodes at most ONE sync wait per
    instruction; split the excess onto NoOp carriers."""
    nsplit = 0
    for f in nc.m.functions:
        for bb in f.blocks:
            il = bb.instructions
            out = []
            for inst in il:
                si = inst.sync_info
                waits = list(si.on_wait) if si and si.on_wait else []
                k = 0
                while len(waits) > max_waits:
                    head, waits = waits[:max_waits], waits[max_waits:]
                    out.append(
                        mybir.InstNoOp(
                            name=f"{inst.name}-waitsplit{k}",
                            engine=inst.engine,
                            ins=[],
                            outs=[],
                            sync_info=mybir.SyncInfo(on_wait=head, on_update=[]),
                        )
                    )
                    k += 1
                    nsplit += 1
                if k:
                    inst.sync_info = mybir.SyncInfo(
                        on_wait=waits,
                        on_update=list(si.on_update) if si.on_update else [],
                    )
                out.append(inst)
            il[:] = out
    return nsplit


def build_nc(n_iters=N_ITERS, split=True):
    sched, (u0_scale, out_scale, g2_0) = _schedule(n_iters)
    nc = bass.Bass(trn_type="TRN2")

    d_in32 = {
        name: nc.dram_tensor(name, [P, FREE], F32, kind="ExternalInput")
        for name in ("u0", "nf")
    }
    d_in16 = {
        name: nc.dram_tensor(name, [P, FREE], F16, kind="ExternalInput")
        for name in ("vb0", "lx", "ly")
    }
    d_out = nc.dram_tensor("out", [P, FREE], F32, kind="ExternalOutput")
    mat_packed, mat_order, i32 = _make_matrices()
    d_mats = nc.inline_tensor(mat_packed, name="mats")
    d_i32 = nc.inline_tensor(i32, name="i32")

    with TileContext(nc) as tc:
        with (
            tc.tile_pool(name="state", bufs=1) as state,
            tc.tile_pool(name="scratch", bufs=4) as scratch,
            tc.tile_pool(name="psA", bufs=2, space="PSUM") as psA,
            tc.tile_pool(name="psB", bufs=2, space="PSUM") as psB,
        ):
            UUs = [state.tile([P, FREE], F32R, name=f"UU{i}")
                   for i in range(2)]
            UU2s = [state.tile([P, FREE], F32, name=f"UU2{i}")
                    for i in range(2)]
            MI32 = state.tile([P, P], F32R, name="MI32")
            VBs = [state.tile([P, FREE], F16, name=f"VB{i}") for i in range(2)]
            Ps = [state.tile([P, FREE + 4], F16, name=f"Pd{i}") for i in range(2)]
            Qs = [state.tile([P, FREE], F16, name=f"Qd{i}") for i in range(2)]
            LX = state.tile([P, FREE], F16, name="LX")
            LY = state.tile([P, FREE], F16, name="LY")
            NLX = state.tile([P, FREE], F16, name="NLX")
            NLY = state.tile([P, FREE], F16, name="NLY")
            NF = state.tile([P, FREE], F32, name="NF")
            MATS = state.tile([P, 6 * P], F16, name="MATS")

            def MAT(name):
                k = mat_order[name]
                return MATS[:, k * P:(k + 1) * P]

            # ---- setup
            nc.gpsimd.dma_start(out=UUs[0], in_=d_in32["u0"].ap())
            nc.gpsimd.dma_start(out=MI32, in_=d_i32.ap())
            nc.sync.dma_start(out=NF, in_=d_in32["nf"].ap())
            nc.gpsimd.dma_start(out=VBs[0], in_=d_in16["vb0"].ap())
            nc.gpsimd.dma_start(out=LX, in_=d_in16["lx"].ap())
            nc.gpsimd.dma_start(out=LY, in_=d_in16["ly"].ap())
            nc.sync.dma_start(out=MATS, in_=d_mats.ap())
            nc.scalar.mul(UU2s[0], UUs[0].bitcast(F32), float(g2_0))
            nc.scalar.mul(NLX, LX, -1.0)
            nc.scalar.mul(NLY, LY, -1.0)
            nc.vector.memset(Ps[0].bitcast(F32), 0.0)
            nc.vector.memset(Ps[1].bitcast(F32), 0.0)
            nc.vector.memset(Qs[0].bitcast(F32), 0.0)

            OutT = state.tile([P, FREE], F32, name="OutT")
            OUT_HALVES = [OutT[:, 0:H], OutT[:, H:FREE]]

            def mm(out, lhsT, rhs, start, stop):
                nc.tensor.matmul(
                    out, lhsT, rhs, start=start, stop=stop,
                    skip_group_check=True,
                )

            def clip(out, val, lam, nlam, mintag):
                if USE_FUSED_CLIP and CLIP_OP is not None:
                    nc.vector._custom_dve(CLIP_OP, out=out, in0=val, in1=lam)
                else:
                    Mn = scratch.tile([P, H], F16, name=mintag, tag=mintag)
                    n = val.shape[-1]
                    nc.vector.tensor_tensor(Mn[:, 0:n], val, lam, AOP.min)
                    nc.vector.tensor_tensor(out, Mn[:, 0:n], nlam, AOP.max)

            for i in range(n_iters):
                m1_n, w_n, w2_n = sched[i]
                a, b = i % 2, (i + 1) % 2
                UUc, UUn = UUs[a], UUs[b]
                UU2c, UU2n = UU2s[a], UU2s[b]
                VBc, VBn = VBs[a], VBs[b]
                Pc, Pn = Ps[a], Ps[b]
                Qc, Qn = Qs[a], Qs[b]

                PSq = [None, None]
                for x in (0, 1):
                    lo, hi = x * H, (x + 1) * H
                    # ---- dual q (partition-dim, PE)
                    PSq[x] = psA.tile([P, H], F32, name=f"PSq{x}",
                                      tag=f"PSq{x}")
                    mm(PSq[x], MAT("mI"), Qc[:, lo:hi], start=True, stop=False)
                    mm(PSq[x], MAT("mLy"), VBc[:, lo:hi], start=False,
                       stop=(x == 1))
                    if x == 0:
                        mm(PSq[x], MAT("mEy"), VBc[:, H:FREE], start=False,
                           stop=True)

                    # ---- dual p (free-dim shifts, DVE fp16 2x)
                    G = scratch.tile([P, H], F16, name=f"G{x}", tag=f"G{x}")
                    nc.vector.tensor_sub(
                        G[:, 0:255], VBc[:, lo + 1:hi], VBc[:, lo:hi - 1]
                    )
                    Ppre = scratch.tile([P, H], F16, name=f"Pp{x}",
                                        tag=f"Pp{x}")
                    nc.vector.tensor_add(
                        Ppre[:, 0:255], G[:, 0:255], Pc[:, 1 + lo:hi]
                    )
                    clip(Pn[:, 1 + lo:hi], Ppre[:, 0:255], LX[:, lo:hi - 1],
                         NLX[:, lo:hi - 1], f"Pm{x}")
                    # per-chunk col 255 of Pn stays 0 (zeroed at setup).

                for x in (0, 1):
                    lo, hi = x * H, (x + 1) * H
                    # ---- q clip (reads PSUM)
                    clip(Qn[:, lo:hi], PSq[x], LY[:, lo:hi], NLY[:, lo:hi],
                         f"Qm{x}")

                for x in (0, 1):
                    lo, hi = x * H, (x + 1) * H
                    # ---- primal accumulation: PS = D - f
                    PSu = psB.tile([P, H], F32, name=f"PSu{x}", tag=f"PSu{x}")
                    if PRELOAD == "act":
                        nc.scalar.copy(PSu, NF[:, lo:hi])
                        first = False
                    else:
                        mm(PSu, MAT("mI"), NF[:, lo:hi], start=True,
                           stop=False)
                        first = False
                    mm(PSu, MI32, UUc[:, lo:hi], start=False, stop=False)
                    mm(PSu, MAT("mI"), Pn[:, lo:hi], start=False, stop=False)
                    mm(PSu, MAT("mNI"), Pn[:, 1 + lo:1 + hi], start=False,
                       stop=False)
                    mm(PSu, MAT("mMyT"), Qn[:, lo:hi], start=False,
                       stop=(x == 0))
                    if x == 1:
                        mm(PSu, MAT("mEyT"), Qn[:, 0:H], start=False,
                           stop=True)

                    if i + 1 < n_iters:
                        # ---- extrapolation: fp16 VB straight off the STT
                        nc.vector.scalar_tensor_tensor(
                            out=VBn[:, lo:hi], in0=PSu, scalar=m1_n,
                            in1=UU2c[:, lo:hi], op0=AOP.mult, op1=AOP.add,
                        )
                        # ---- primal-state rescales on ACT (full-iter slack)
                        nc.scalar.mul(UUn[:, lo:hi], PSu, float(w_n))
                        nc.scalar.mul(UU2n[:, lo:hi], PSu, float(w2_n))
                    else:
                        nc.scalar.mul(OUT_HALVES[x], PSu, float(out_scale))

            nc.sync.dma_start(out=d_out.ap(), in_=OutT)

    nc.finalize()
    if split:
        split_excess_waits(nc)
    return nc


_NC_CACHE = {}


def _get_nc(n_iters=N_ITERS):
    key = n_iters
    if key not in _NC_CACHE:
        _NC_CACHE[key] = build_nc(n_iters)
    return _NC_CACHE[key]


def kernel(f, lam):
    from concourse.bass_utils import run_bass_kernel_spmd

    f = np.asarray(f, dtype=np.float32)
    lam = np.asarray(lam, dtype=np.float32)
    nc = _get_nc()
    in_maps = [_per_core_inputs(f[b], lam[b]) for b in range(B)]
    res = run_bass_kernel_spmd(nc, in_maps, core_ids=list(range(B)))
    return np.stack([_from_layout_a(res.results[b]["out"]) for b in range(B)])


if __name__ == "__main__":
    import sys

    if "--build" in sys.argv:
        import time

        t0 = time.time()
        nc = build_nc(int(sys.argv[sys.argv.index("--build") + 1])
                      if len(sys.argv) > 2 else N_ITERS)
        print(f"build ok in {time.time()-t0:.1f}s")


# revision 21
# speedup vs baseline: 1.3178x; 1.0213x over previous
"""Trainium2 Bass kernel for the DifferentiableTVLayer PDHG solve.

Accelerated Chambolle-Pock (strong-convexity schedule, gamma=0.3) brings the
iteration count from 200 to N_ITERS while staying inside the 2e-2 relative
error budget. The primal state is kept pre-scaled (U~ = u / s_n with
s_{n+1} = s_n/(1+tau_n)) so the contraction and the tau_n*f term disappear
from the matmul path: the difference-operator matrices are constant +-1,
schedule dependence rides on STT immediates and on per-iteration scaled
identity weight tiles (A_n*I / -B_n*I) for the extrapolation.

Per-core layout ("layout A"): SBUF tiles [128, 512] where
    tile[p, c*256 + h] = X[h, w]  with  w = c*128 + p,  c in {0,1}.
The image is processed as two W-chunk halves (free cols [0:256) / [256:512)).

Per iteration n (per half x):
    PSq_x = I@Q_x + Ly@VB_x (+ Ey@VB_B for x=0)            (PE, fp16)
    Q'_x  = clip(PSq_x, +-LY)                              (DVE fused clip)
    P'_x  = clip(P + dxF(VB), +-LX)                        (DVE: sub, add, clip)
    PSu_x = (-f preload, ACT) + I@Ppad - I@Ppad' + MyT@Q' (+EyT@Q'_A) = D - f
    U~'   = (PSu * c_n) + U~                               (DVE STT)
    PSv_x = (A_n I)@U~' + (-B_n I)@U~                      (PE, per-iter weights)
    VB'   = copy(PSv_x) -> fp16                            (ACT)
"""

import numpy as np

import concourse.bass as bass
import concourse.mybir as mybir
from concourse.tile import TileContext

B, H, W = 8, 256, 256
P, NCH = 128, 2
FREE = NCH * H  # 512

N_ITERS = 40
TAU0 = 0.35355339
SIGMA0 = 1.0 / (8.0 * TAU0)
GAMMA = 0.3

F32 = mybir.dt.float32
F32R = mybir.dt.float32r
F16 = mybir.dt.float16
AOP = mybir.AluOpType

USE_FUSED_CLIP = False
PRELOAD = "act"   # "act" | "mm"


def _schedule(n_iters=N_ITERS):
    """Per-iteration scalars for the UU-state (raw PSu copy) formulation.

    UU(n) = U~_n / c_n lives in SBUF; PSu(n) = UU(n) + D - f (U injected by
    a constant identity matmul); U~_{n+1} = c_n PSu(n).
      VB'   = A c_n PSu - B c_n UU = (PSu * m1) + UU2   (DVE STT, fp16 out)
      UU'   = w  * PSu   (ACT scale-copy)
      UU2'  = w2 * PSu   (ACT scale-copy; UU2 = g2*UU pre-scaled)
    Returns per-iter (m1, w, w2) plus (u0_scale, out_scale, g2_0).
    """
    tau, sigma, s = TAU0, SIGMA0, 1.0
    cs, ABs = [], []
    for _ in range(n_iters):
        theta = 1.0 / np.sqrt(1.0 + 2.0 * GAMMA * tau)
        s_next = s / (1.0 + tau)
        sig_next = sigma / theta
        cs.append(-tau / s)
        A_n = sig_next * (1.0 + theta) * s_next
        B_n = sig_next * theta * s
        ABs.append((A_n, B_n))
        tau, sigma, s = tau * theta, sig_next, s_next
    s_final = s
    g2s = [-ABs[i][1] * cs[i] for i in range(n_iters)]
    out = []
    for i in range(n_iters):
        A_n, B_n = ABs[i]
        m1 = A_n * cs[i]                      # VB' = m1*PSu + UU2
        w = cs[i] / cs[i + 1] if i + 1 < n_iters else 0.0
        w2 = g2s[i + 1] * w if i + 1 < n_iters else 0.0
        out.append((float(m1), float(w), float(w2)))
    u0_scale = 1.0 / cs[0]
    out_scale = s_final * cs[-1]
    return out, (float(u0_scale), float(out_scale), float(g2s[0]))


SCHED, SCALES = _schedule()


# ---------------------------------------------------------------- host layout
def _to_layout_a(x):
    """[H, W] -> [128, 512]: out[p, c*256+h] = x[h, c*128+p]."""
    return np.ascontiguousarray(
        x.T.reshape(NCH, P, H).transpose(1, 0, 2).reshape(P, FREE)
    )


def _from_layout_a(t):
    return np.ascontiguousarray(
        t.reshape(P, NCH, H).transpose(1, 0, 2).reshape(W, H).T
    )


def _make_matrices():
    """Constant lhsT operators [k, m]: out[m] = sum_k lhsT[k,m] rhs[k].
    Packed side by side into one [128, 5*128] fp16 tensor."""
    Ly = np.zeros((P, P), np.float16)
    for m in range(P):
        Ly[m, m] = -1.0
        if m + 1 < P:
            Ly[m + 1, m] = 1.0
    Ey = np.zeros((P, P), np.float16)
    Ey[0, 127] = 1.0
    MyT = np.zeros((P, P), np.float16)   # PS accumulates +dyT q
    for m in range(P):
        MyT[m, m] = -1.0
        if m - 1 >= 0:
            MyT[m - 1, m] = 1.0
    EyT = np.zeros((P, P), np.float16)
    EyT[127, 0] = 1.0
    I = np.eye(P, dtype=np.float16)
    NI = (-I).astype(np.float16)
    packed = np.concatenate([I, NI, Ly, Ey, MyT, EyT], axis=1)
    order = {"mI": 0, "mNI": 1, "mLy": 2, "mEy": 3, "mMyT": 4, "mEyT": 5}
    return packed, order, np.eye(P, dtype=np.float32)


def _make_ab_weights(sched):
    """Per-iteration (A_n*I | -B_n*I) fp16 weight blocks, packed."""
    n = len(sched)
    out = np.zeros((P, 2 * n * P), np.float32)
    for i, (_, A_n, B_n) in enumerate(sched):
        out[:, (2 * i) * P:(2 * i + 1) * P] = A_n * np.eye(P)
        out[:, (2 * i + 1) * P:(2 * i + 2) * P] = -B_n * np.eye(P)
    return out


def _per_core_inputs(f_img, lam_img):
    fa = _to_layout_a(f_img).astype(np.float32)

    lamx = np.concatenate([lam_img[1:, :], np.zeros((1, W), np.float32)])
    lx3 = _to_layout_a(lamx).reshape(P, NCH, H).copy()
    lx3[:, :, 255] = 0.0
    lx = lx3.reshape(P, FREE).astype(np.float16)

    lamy = np.concatenate([lam_img[:, 1:], np.zeros((H, 1), np.float32)], axis=1)
    ly = _to_layout_a(lamy).astype(np.float16)  # (c=1, p=127) col already zero

    u0s = SCALES[0]
    return {
        "u0": np.ascontiguousarray(u0s * fa),
        "nf": np.ascontiguousarray(-fa),
        "vb0": (SIGMA0 * fa).astype(np.float16),
        "lx": np.ascontiguousarray(lx),
        "ly": np.ascontiguousarray(ly),
    }


# ---------------------------------------------------------------- custom op
def _register_clip_op():
    """out = clip(in0, -in1, +in1) as a single DVE instruction."""
    from concourse import dve_ops
    from concourse.dve_spec import Spec, Src0, Src1, maxx, minn, lower
    from concourse.dve_uop import DveOpSpec

    for op in dve_ops.OPS:
        if op.name == "TV_CLIP2_ANT":
            return op
    spec = Spec(
        body=minn(maxx(Src0, -Src1), Src1),
        reference=lambda in0, in1, s0, s1, imm2: np.minimum(
            np.maximum(in0, -in1), in1
        ).astype(np.float32),
    )
    op = dve_ops.DveOp("TV_CLIP2_ANT", spec, subdim=False, uops_sha={})
    dve_ops.OPS.append(op)
    dve_ops.CUSTOM_DVE_SPECS[op.name] = spec
    dve_ops._SUB_OPCODE_FOR_NAME[op.name] = (
        max(dve_ops._SUB_OPCODE_FOR_NAME.values()) + 1
    )
    for ver in ("v3", "v4"):
        try:
            s = DveOpSpec(
                name=op.name,
                opcode=dve_ops.get_dve_sub_opcode(op.name),
                uops=lower(spec, ver=ver),
                rd1_en=True,
            )
            op.uops_sha[ver] = s.sha(ver)
        except Exception:
            pass
    return op


try:
    CLIP_OP = _register_clip_op()
except Exception:
    CLIP_OP = None


# ---------------------------------------------------------------- bass build
def split_excess_waits(nc, max_waits=1):
    """This neuronxcc/walrus build encodes at most ONE sync wait per
    instruction; split the excess onto NoOp carriers."""
    nsplit = 0
    for f in nc.m.functions:
        for bb in f.blocks:
            il = bb.instructions
            out = []
            for inst in il:
                si = inst.sync_info
                waits = list(si.on_wait) if si and si.on_wait else []
                k = 0
                while len(waits) > max_waits:
                    head, waits = waits[:max_waits], waits[max_waits:]
                    out.append(
                        mybir.InstNoOp(
                            name=f"{inst.name}-waitsplit{k}",
                            engine=inst.engine,
                            ins=[],
                            outs=[],
                            sync_info=mybir.SyncInfo(on_wait=head, on_update=[]),
                        )
                    )
                    k += 1
                    nsplit += 1
                if k:
                    inst.sync_info = mybir.SyncInfo(
                        on_wait=waits,
                        on_update=list(si.on_update) if si.on_update else [],
                    )
                out.append(inst)
            il[:] = out
    return nsplit


def build_nc(n_iters=N_ITERS, split=True):
    sched, (u0_scale, out_scale, g2_0) = _schedule(n_iters)
    nc = bass.Bass(trn_type="TRN2")

    d_in32 = {
        name: nc.dram_tensor(name, [P, FREE], F32, kind="ExternalInput")
        for name in ("u0", "nf")
    }
    d_in16 = {
        name: nc.dram_tensor(name, [P, FREE], F16, kind="ExternalInput")
        for name in ("vb0", "lx", "ly")
    }
    d_out = nc.dram_tensor("out", [P, FREE], F32, kind="ExternalOutput")
    mat_packed, mat_order, i32 = _make_matrices()
    d_mats = nc.inline_tensor(mat_packed, name="mats")
    d_i32 = nc.inline_tensor(i32, name="i32")

    with TileContext(nc) as tc:
        with (
            tc.tile_pool(name="state", bufs=1) as state,
            tc.tile_pool(name="scratch", bufs=4) as scratch,
            tc.tile_pool(name="psA", bufs=1, space="PSUM") as psA,
            tc.tile_pool(name="psB", bufs=2, space="PSUM") as psB,
        ):
            UUs = [state.tile([P, FREE], F32R, name=f"UU{i}")
                   for i in range(2)]
            UU2s = [state.tile([P, FREE], F32, name=f"UU2{i}")
                    for i in range(2)]
            MI32 = state.tile([P, P], F32R, name="MI32")
            VBs = [state.tile([P, FREE], F16, name=f"VB{i}") for i in range(2)]
            Ps = [state.tile([P, FREE + 4], F16, name=f"Pd{i}") for i in range(2)]
            Qs = [state.tile([P, FREE], F16, name=f"Qd{i}") for i in range(2)]
            LX = state.tile([P, FREE], F16, name="LX")
            LY = state.tile([P, FREE], F16, name="LY")
            NLX = state.tile([P, FREE], F16, name="NLX")
            NLY = state.tile([P, FREE], F16, name="NLY")
            NF = state.tile([P, FREE], F32R, name="NF")
            MATS = state.tile([P, 6 * P], F16, name="MATS")

            def MAT(name):
                k = mat_order[name]
                return MATS[:, k * P:(k + 1) * P]

            # ---- setup
            nc.gpsimd.dma_start(out=UUs[0], in_=d_in32["u0"].ap())
            nc.gpsimd.dma_start(out=MI32, in_=d_i32.ap())
            nc.gpsimd.dma_start(out=NF, in_=d_in32["nf"].ap())
            nc.gpsimd.dma_start(out=VBs[0], in_=d_in16["vb0"].ap())
            nc.gpsimd.dma_start(out=LX, in_=d_in16["lx"].ap())
            nc.gpsimd.dma_start(out=LY, in_=d_in16["ly"].ap())
            nc.sync.dma_start(out=MATS, in_=d_mats.ap())
            nc.scalar.mul(UU2s[0], UUs[0].bitcast(F32), float(g2_0))
            nc.scalar.mul(NLX, LX, -1.0)
            nc.scalar.mul(NLY, LY, -1.0)
            nc.vector.memset(Ps[0].bitcast(F32), 0.0)
            nc.vector.memset(Ps[1].bitcast(F32), 0.0)
            nc.vector.memset(Qs[0].bitcast(F32), 0.0)

            OutT = state.tile([P, FREE], F32, name="OutT")
            OUT_HALVES = [OutT[:, 0:H], OutT[:, H:FREE]]

            def mm(out, lhsT, rhs, start, stop):
                nc.tensor.matmul(
                    out, lhsT, rhs, start=start, stop=stop,
                    skip_group_check=True,
                )

            def clip(out, val, lam, nlam, mintag):
                if USE_FUSED_CLIP and CLIP_OP is not None:
                    nc.vector._custom_dve(CLIP_OP, out=out, in0=val, in1=lam)
                else:
                    Mn = scratch.tile([P, H], F16, name=mintag, tag=mintag)
                    n = val.shape[-1]
                    nc.vector.tensor_tensor(Mn[:, 0:n], val, lam, AOP.min)
                    nc.vector.tensor_tensor(out, Mn[:, 0:n], nlam, AOP.max)

            for i in range(n_iters):
                m1_n, w_n, w2_n = sched[i]
                a, b = i % 2, (i + 1) % 2
                UUc, UUn = UUs[a], UUs[b]
                UU2c, UU2n = UU2s[a], UU2s[b]
                VBc, VBn = VBs[a], VBs[b]
                Pc, Pn = Ps[a], Ps[b]
                Qc, Qn = Qs[a], Qs[b]

                PSq = [None, None]
                for x in (0, 1):
                    lo, hi = x * H, (x + 1) * H
                    # ---- dual q (partition-dim, PE)
                    PSq[x] = psA.tile([P, H], F32, name=f"PSq{x}",
                                      tag=f"PSq{x}")
                    mm(PSq[x], MAT("mI"), Qc[:, lo:hi], start=True, stop=False)
                    mm(PSq[x], MAT("mLy"), VBc[:, lo:hi], start=False,
                       stop=(x == 1))
                    if x == 0:
                        mm(PSq[x], MAT("mEy"), VBc[:, H:FREE], start=False,
                           stop=True)

                    # ---- dual p: dx on DVE, p-add on PE, clip reads PSUM
                    G = scratch.tile([P, H], F16, name=f"G{x}", tag=f"G{x}")
                    nc.vector.tensor_sub(
                        G[:, 0:255], VBc[:, lo + 1:hi], VBc[:, lo:hi - 1]
                    )
                    PSp = psA.tile([P, H], F32, name=f"PSp{x}", tag=f"PSp{x}")
                    mm(PSp[:, 0:255], MAT("mI"), G[:, 0:255], start=True,
                       stop=False)
                    mm(PSp[:, 0:255], MAT("mI"), Pc[:, 1 + lo:hi],
                       start=False, stop=True)
                    clip(Pn[:, 1 + lo:hi], PSp[:, 0:255], LX[:, lo:hi - 1],
                         NLX[:, lo:hi - 1], f"Pm{x}")
                    # per-chunk col 255 of Pn stays 0 (zeroed at setup).

                for x in (0, 1):
                    lo, hi = x * H, (x + 1) * H
                    # ---- q clip (reads PSUM)
                    clip(Qn[:, lo:hi], PSq[x], LY[:, lo:hi], NLY[:, lo:hi],
                         f"Qm{x}")

                for x in (0, 1):
                    lo, hi = x * H, (x + 1) * H
                    # ---- primal accumulation: PS = D - f
                    PSu = psB.tile([P, H], F32, name=f"PSu{x}", tag=f"PSu{x}")
                    mm(PSu, MI32, NF[:, lo:hi], start=True, stop=False)
                    mm(PSu, MI32, UUc[:, lo:hi], start=False, stop=False)
                    mm(PSu, MAT("mI"), Pn[:, lo:hi], start=False, stop=False)
                    mm(PSu, MAT("mNI"), Pn[:, 1 + lo:1 + hi], start=False,
                       stop=False)
                    mm(PSu, MAT("mMyT"), Qn[:, lo:hi], start=False,
                       stop=(x == 0))
                    if x == 1:
                        mm(PSu, MAT("mEyT"), Qn[:, 0:H], start=False,
                           stop=True)

                    if i + 1 < n_iters:
                        # ---- extrapolation: fp16 VB straight off the STT
                        nc.vector.scalar_tensor_tensor(
                            out=VBn[:, lo:hi], in0=PSu, scalar=m1_n,
                            in1=UU2c[:, lo:hi], op0=AOP.mult, op1=AOP.add,
                        )
                        # ---- primal-state rescales on ACT (full-iter slack)
                        nc.scalar.mul(UUn[:, lo:hi], PSu, float(w_n))
                        nc.scalar.mul(UU2n[:, lo:hi], PSu, float(w2_n))
                    else:
                        nc.scalar.mul(OUT_HALVES[x], PSu, float(out_scale))

            nc.sync.dma_start(out=d_out.ap(), in_=OutT)

    nc.finalize()
    if split:
        split_excess_waits(nc)
    return nc


_NC_CACHE = {}


def _get_nc(n_iters=N_ITERS):
    key = n_iters
    if key not in _NC_CACHE:
        _NC_CACHE[key] = build_nc(n_iters)
    return _NC_CACHE[key]


def kernel(f, lam):
    from concourse.bass_utils import run_bass_kernel_spmd

    f = np.asarray(f, dtype=np.float32)
    lam = np.asarray(lam, dtype=np.float32)
    nc = _get_nc()
    in_maps = [_per_core_inputs(f[b], lam[b]) for b in range(B)]
    res = run_bass_kernel_spmd(nc, in_maps, core_ids=list(range(B)))
    return np.stack([_from_layout_a(res.results[b]["out"]) for b in range(B)])


if __name__ == "__main__":
    import sys

    if "--build" in sys.argv:
        import time

        t0 = time.time()
        nc = build_nc(int(sys.argv[sys.argv.index("--build") + 1])
                      if len(sys.argv) > 2 else N_ITERS)
        print(f"build ok in {time.time()-t0:.1f}s")


# revision 22
# speedup vs baseline: 1.4104x; 1.0702x over previous
"""Trainium2 Bass kernel for the DifferentiableTVLayer PDHG solve.

Accelerated Chambolle-Pock (strong-convexity schedule, gamma=0.3) brings the
iteration count from 200 to N_ITERS while staying inside the 2e-2 relative
error budget. The primal state is kept pre-scaled (U~ = u / s_n with
s_{n+1} = s_n/(1+tau_n)) so the contraction and the tau_n*f term disappear
from the matmul path: the difference-operator matrices are constant +-1,
schedule dependence rides on STT immediates and on per-iteration scaled
identity weight tiles (A_n*I / -B_n*I) for the extrapolation.

Per-core layout ("layout A"): SBUF tiles [128, 512] where
    tile[p, c*256 + h] = X[h, w]  with  w = c*128 + p,  c in {0,1}.
The image is processed as two W-chunk halves (free cols [0:256) / [256:512)).

Per iteration n (per half x):
    PSq_x = I@Q_x + Ly@VB_x (+ Ey@VB_B for x=0)            (PE, fp16)
    Q'_x  = clip(PSq_x, +-LY)                              (DVE fused clip)
    P'_x  = clip(P + dxF(VB), +-LX)                        (DVE: sub, add, clip)
    PSu_x = (-f preload, ACT) + I@Ppad - I@Ppad' + MyT@Q' (+EyT@Q'_A) = D - f
    U~'   = (PSu * c_n) + U~                               (DVE STT)
    PSv_x = (A_n I)@U~' + (-B_n I)@U~                      (PE, per-iter weights)
    VB'   = copy(PSv_x) -> fp16                            (ACT)
"""

import numpy as np

import concourse.bass as bass
import concourse.mybir as mybir
from concourse.tile import TileContext

B, H, W = 8, 256, 256
P, NCH = 128, 2
FREE = NCH * H  # 512

N_ITERS = 40
TAU0 = 0.35355339
SIGMA0 = 1.0 / (8.0 * TAU0)
GAMMA = 0.3

F32 = mybir.dt.float32
F32R = mybir.dt.float32r
F16 = mybir.dt.float16
AOP = mybir.AluOpType

USE_FUSED_CLIP = False
PRELOAD = "act"   # "act" | "mm"


def _schedule(n_iters=N_ITERS):
    """Per-iteration scalars for the UU-state (raw PSu copy) formulation.

    UU(n) = U~_n / c_n lives in SBUF; PSu(n) = UU(n) + D - f (U injected by
    a constant identity matmul); U~_{n+1} = c_n PSu(n).
      VB'   = A c_n PSu - B c_n UU = (PSu * m1) + UU2   (DVE STT, fp16 out)
      UU'   = w  * PSu   (ACT scale-copy)
      UU2'  = w2 * PSu   (ACT scale-copy; UU2 = g2*UU pre-scaled)
    Returns per-iter (m1, w, w2) plus (u0_scale, out_scale, g2_0).
    """
    tau, sigma, s = TAU0, SIGMA0, 1.0
    cs, ABs = [], []
    for _ in range(n_iters):
        theta = 1.0 / np.sqrt(1.0 + 2.0 * GAMMA * tau)
        s_next = s / (1.0 + tau)
        sig_next = sigma / theta
        cs.append(-tau / s)
        A_n = sig_next * (1.0 + theta) * s_next
        B_n = sig_next * theta * s
        ABs.append((A_n, B_n))
        tau, sigma, s = tau * theta, sig_next, s_next
    s_final = s
    g2s = [-ABs[i][1] * cs[i] for i in range(n_iters)]
    out = []
    for i in range(n_iters):
        A_n, B_n = ABs[i]
        m1 = A_n * cs[i]                      # VB' = m1*PSu + UU2
        w = cs[i] / cs[i + 1] if i + 1 < n_iters else 0.0
        w2 = g2s[i + 1] * w if i + 1 < n_iters else 0.0
        out.append((float(m1), float(w), float(w2)))
    u0_scale = 1.0 / cs[0]
    out_scale = s_final * cs[-1]
    return out, (float(u0_scale), float(out_scale), float(g2s[0]))


SCHED, SCALES = _schedule()


# ---------------------------------------------------------------- host layout
def _to_layout_a(x):
    """[H, W] -> [128, 512]: out[p, c*256+h] = x[h, c*128+p]."""
    return np.ascontiguousarray(
        x.T.reshape(NCH, P, H).transpose(1, 0, 2).reshape(P, FREE)
    )


def _from_layout_a(t):
    return np.ascontiguousarray(
        t.reshape(P, NCH, H).transpose(1, 0, 2).reshape(W, H).T
    )


def _make_matrices():
    """Constant lhsT operators [k, m]: out[m] = sum_k lhsT[k,m] rhs[k].
    Packed side by side into one [128, 5*128] fp16 tensor."""
    Ly = np.zeros((P, P), np.float16)
    for m in range(P):
        Ly[m, m] = -1.0
        if m + 1 < P:
            Ly[m + 1, m] = 1.0
    Ey = np.zeros((P, P), np.float16)
    Ey[0, 127] = 1.0
    MyT = np.zeros((P, P), np.float16)   # PS accumulates +dyT q
    for m in range(P):
        MyT[m, m] = -1.0
        if m - 1 >= 0:
            MyT[m - 1, m] = 1.0
    EyT = np.zeros((P, P), np.float16)
    EyT[127, 0] = 1.0
    I = np.eye(P, dtype=np.float16)
    NI = (-I).astype(np.float16)
    packed = np.concatenate([I, NI, Ly, Ey, MyT, EyT], axis=1)
    order = {"mI": 0, "mNI": 1, "mLy": 2, "mEy": 3, "mMyT": 4, "mEyT": 5}
    return packed, order, np.eye(P, dtype=np.float32)


def _make_ab_weights(sched):
    """Per-iteration (A_n*I | -B_n*I) fp16 weight blocks, packed."""
    n = len(sched)
    out = np.zeros((P, 2 * n * P), np.float32)
    for i, (_, A_n, B_n) in enumerate(sched):
        out[:, (2 * i) * P:(2 * i + 1) * P] = A_n * np.eye(P)
        out[:, (2 * i + 1) * P:(2 * i + 2) * P] = -B_n * np.eye(P)
    return out


def _per_core_inputs(f_img, lam_img):
    fa = _to_layout_a(f_img).astype(np.float32)

    lamx = np.concatenate([lam_img[1:, :], np.zeros((1, W), np.float32)])
    lx3 = _to_layout_a(lamx).reshape(P, NCH, H).copy()
    lx3[:, :, 255] = 0.0
    lx = lx3.reshape(P, FREE).astype(np.float16)

    lamy = np.concatenate([lam_img[:, 1:], np.zeros((H, 1), np.float32)], axis=1)
    ly = _to_layout_a(lamy).astype(np.float16)  # (c=1, p=127) col already zero

    u0s = SCALES[0]
    return {
        "u0": np.ascontiguousarray(u0s * fa),
        "nf": np.ascontiguousarray(-fa),
        "vb0": (SIGMA0 * fa).astype(np.float16),
        "lx": np.ascontiguousarray(lx),
        "ly": np.ascontiguousarray(ly),
    }


# ---------------------------------------------------------------- custom op
def _register_clip_op():
    """out = clip(in0, -in1, +in1) as a single DVE instruction."""
    from concourse import dve_ops
    from concourse.dve_spec import Spec, Src0, Src1, maxx, minn, lower
    from concourse.dve_uop import DveOpSpec

    for op in dve_ops.OPS:
        if op.name == "TV_CLIP2_ANT":
            return op
    spec = Spec(
        body=minn(maxx(Src0, -Src1), Src1),
        reference=lambda in0, in1, s0, s1, imm2: np.minimum(
            np.maximum(in0, -in1), in1
        ).astype(np.float32),
    )
    op = dve_ops.DveOp("TV_CLIP2_ANT", spec, subdim=False, uops_sha={})
    dve_ops.OPS.append(op)
    dve_ops.CUSTOM_DVE_SPECS[op.name] = spec
    dve_ops._SUB_OPCODE_FOR_NAME[op.name] = (
        max(dve_ops._SUB_OPCODE_FOR_NAME.values()) + 1
    )
    for ver in ("v3", "v4"):
        try:
            s = DveOpSpec(
                name=op.name,
                opcode=dve_ops.get_dve_sub_opcode(op.name),
                uops=lower(spec, ver=ver),
                rd1_en=True,
            )
            op.uops_sha[ver] = s.sha(ver)
        except Exception:
            pass
    return op


try:
    CLIP_OP = _register_clip_op()
except Exception:
    CLIP_OP = None


# ---------------------------------------------------------------- bass build
def split_excess_waits(nc, max_waits=1):
    """This neuronxcc/walrus build encodes at most ONE sync wait per
    instruction; split the excess onto NoOp carriers."""
    nsplit = 0
    for f in nc.m.functions:
        for bb in f.blocks:
            il = bb.instructions
            out = []
            for inst in il:
                si = inst.sync_info
                waits = list(si.on_wait) if si and si.on_wait else []
                k = 0
                while len(waits) > max_waits:
                    head, waits = waits[:max_waits], waits[max_waits:]
                    out.append(
                        mybir.InstNoOp(
                            name=f"{inst.name}-waitsplit{k}",
                            engine=inst.engine,
                            ins=[],
                            outs=[],
                            sync_info=mybir.SyncInfo(on_wait=head, on_update=[]),
                        )
                    )
                    k += 1
                    nsplit += 1
                if k:
                    inst.sync_info = mybir.SyncInfo(
                        on_wait=waits,
                        on_update=list(si.on_update) if si.on_update else [],
                    )
                out.append(inst)
            il[:] = out
    return nsplit


def build_nc(n_iters=N_ITERS, split=True):
    sched, (u0_scale, out_scale, g2_0) = _schedule(n_iters)
    nc = bass.Bass(trn_type="TRN2")

    d_in32 = {
        name: nc.dram_tensor(name, [P, FREE], F32, kind="ExternalInput")
        for name in ("u0", "nf")
    }
    d_in16 = {
        name: nc.dram_tensor(name, [P, FREE], F16, kind="ExternalInput")
        for name in ("vb0", "lx", "ly")
    }
    d_out = nc.dram_tensor("out", [P, FREE], F32, kind="ExternalOutput")
    mat_packed, mat_order, i32 = _make_matrices()
    d_mats = nc.inline_tensor(mat_packed, name="mats")
    d_i32 = nc.inline_tensor(i32, name="i32")

    with TileContext(nc) as tc:
        with (
            tc.tile_pool(name="state", bufs=1) as state,
            tc.tile_pool(name="scratch", bufs=4) as scratch,
            tc.tile_pool(name="psA", bufs=2, space="PSUM") as psA,
            tc.tile_pool(name="psB", bufs=2, space="PSUM") as psB,
        ):
            UUs = [state.tile([P, FREE], F32R, name=f"UU{i}")
                   for i in range(2)]
            UU2s = [state.tile([P, FREE], F32, name=f"UU2{i}")
                    for i in range(2)]
            MI32 = state.tile([P, P], F32R, name="MI32")
            VBs = [state.tile([P, FREE], F16, name=f"VB{i}") for i in range(2)]
            Ps = [state.tile([P, FREE + 4], F16, name=f"Pd{i}") for i in range(2)]
            Qs = [state.tile([P, FREE], F16, name=f"Qd{i}") for i in range(2)]
            LX = state.tile([P, FREE], F16, name="LX")
            LY = state.tile([P, FREE], F16, name="LY")
            NLX = state.tile([P, FREE], F16, name="NLX")
            NLY = state.tile([P, FREE], F16, name="NLY")
            NF = state.tile([P, FREE], F32R, name="NF")
            MATS = state.tile([P, 6 * P], F16, name="MATS")

            def MAT(name):
                k = mat_order[name]
                return MATS[:, k * P:(k + 1) * P]

            # ---- setup
            nc.gpsimd.dma_start(out=UUs[0], in_=d_in32["u0"].ap())
            nc.gpsimd.dma_start(out=MI32, in_=d_i32.ap())
            nc.gpsimd.dma_start(out=NF, in_=d_in32["nf"].ap())
            nc.gpsimd.dma_start(out=VBs[0], in_=d_in16["vb0"].ap())
            nc.gpsimd.dma_start(out=LX, in_=d_in16["lx"].ap())
            nc.gpsimd.dma_start(out=LY, in_=d_in16["ly"].ap())
            nc.sync.dma_start(out=MATS, in_=d_mats.ap())
            nc.scalar.mul(UU2s[0], UUs[0].bitcast(F32), float(g2_0))
            nc.scalar.mul(NLX, LX, -1.0)
            nc.scalar.mul(NLY, LY, -1.0)
            nc.vector.memset(Ps[0].bitcast(F32), 0.0)
            nc.vector.memset(Ps[1].bitcast(F32), 0.0)
            nc.vector.memset(Qs[0].bitcast(F32), 0.0)

            OutT = state.tile([P, FREE], F32, name="OutT")
            OUT_HALVES = [OutT[:, 0:H], OutT[:, H:FREE]]

            def mm(out, lhsT, rhs, start, stop):
                nc.tensor.matmul(
                    out, lhsT, rhs, start=start, stop=stop,
                    skip_group_check=True,
                )

            def clip(out, val, lam, nlam, mintag):
                if USE_FUSED_CLIP and CLIP_OP is not None:
                    nc.vector._custom_dve(CLIP_OP, out=out, in0=val, in1=lam)
                else:
                    Mn = scratch.tile([P, H], F16, name=mintag, tag=mintag)
                    n = val.shape[-1]
                    nc.vector.tensor_tensor(Mn[:, 0:n], val, lam, AOP.min)
                    nc.vector.tensor_tensor(out, Mn[:, 0:n], nlam, AOP.max)

            for i in range(n_iters):
                m1_n, w_n, w2_n = sched[i]
                a, b = i % 2, (i + 1) % 2
                UUc, UUn = UUs[a], UUs[b]
                UU2c, UU2n = UU2s[a], UU2s[b]
                VBc, VBn = VBs[a], VBs[b]
                Pc, Pn = Ps[a], Ps[b]
                Qc, Qn = Qs[a], Qs[b]

                PSq = [None, None]
                for x in (0, 1):
                    lo, hi = x * H, (x + 1) * H
                    # ---- dual q (partition-dim, PE)
                    PSq[x] = psA.tile([P, H], F32, name=f"PSq{x}",
                                      tag=f"PSq{x}")
                    mm(PSq[x], MAT("mI"), Qc[:, lo:hi], start=True, stop=False)
                    mm(PSq[x], MAT("mLy"), VBc[:, lo:hi], start=False,
                       stop=(x == 1))
                    if x == 0:
                        mm(PSq[x], MAT("mEy"), VBc[:, H:FREE], start=False,
                           stop=True)

                    # ---- dual p (free-dim shifts, DVE fp16 2x)
                    G = scratch.tile([P, H], F16, name=f"G{x}", tag=f"G{x}")
                    nc.vector.tensor_sub(
                        G[:, 0:255], VBc[:, lo + 1:hi], VBc[:, lo:hi - 1]
                    )
                    Ppre = scratch.tile([P, H], F16, name=f"Pp{x}",
                                        tag=f"Pp{x}")
                    nc.vector.tensor_add(
                        Ppre[:, 0:255], G[:, 0:255], Pc[:, 1 + lo:hi]
                    )
                    clip(Pn[:, 1 + lo:hi], Ppre[:, 0:255], LX[:, lo:hi - 1],
                         NLX[:, lo:hi - 1], f"Pm{x}")
                    # per-chunk col 255 of Pn stays 0 (zeroed at setup).

                for x in (0, 1):
                    lo, hi = x * H, (x + 1) * H
                    # ---- q clip (reads PSUM)
                    clip(Qn[:, lo:hi], PSq[x], LY[:, lo:hi], NLY[:, lo:hi],
                         f"Qm{x}")

                for x in (0, 1):
                    lo, hi = x * H, (x + 1) * H
                    # ---- primal accumulation: PS = D - f
                    PSu = psB.tile([P, H], F32, name=f"PSu{x}", tag=f"PSu{x}")
                    mm(PSu, MI32, NF[:, lo:hi], start=True, stop=False)
                    mm(PSu, MI32, UUc[:, lo:hi], start=False, stop=False)
                    mm(PSu, MAT("mI"), Pn[:, lo:hi], start=False, stop=False)
                    mm(PSu, MAT("mNI"), Pn[:, 1 + lo:1 + hi], start=False,
                       stop=False)
                    mm(PSu, MAT("mMyT"), Qn[:, lo:hi], start=False,
                       stop=(x == 0))
                    if x == 1:
                        mm(PSu, MAT("mEyT"), Qn[:, 0:H], start=False,
                           stop=True)

                    if i + 1 < n_iters:
                        # ---- extrapolation: fp16 VB straight off the STT
                        nc.vector.scalar_tensor_tensor(
                            out=VBn[:, lo:hi], in0=PSu, scalar=m1_n,
                            in1=UU2c[:, lo:hi], op0=AOP.mult, op1=AOP.add,
                        )
                        # ---- primal-state rescales on ACT (full-iter slack)
                        nc.scalar.mul(UUn[:, lo:hi], PSu, float(w_n))
                        nc.scalar.mul(UU2n[:, lo:hi], PSu, float(w2_n))
                    else:
                        nc.scalar.mul(OUT_HALVES[x], PSu, float(out_scale))

            nc.sync.dma_start(out=d_out.ap(), in_=OutT)

    nc.finalize()
    if split:
        split_excess_waits(nc)
    return nc


_NC_CACHE = {}


def _get_nc(n_iters=N_ITERS):
    key = n_iters
    if key not in _NC_CACHE:
        _NC_CACHE[key] = build_nc(n_iters)
    return _NC_CACHE[key]


def kernel(f, lam):
    from concourse.bass_utils import run_bass_kernel_spmd

    f = np.asarray(f, dtype=np.float32)
    lam = np.asarray(lam, dtype=np.float32)
    nc = _get_nc()
    in_maps = [_per_core_inputs(f[b], lam[b]) for b in range(B)]
    res = run_bass_kernel_spmd(nc, in_maps, core_ids=list(range(B)))
    return np.stack([_from_layout_a(res.results[b]["out"]) for b in range(B)])


if __name__ == "__main__":
    import sys

    if "--build" in sys.argv:
        import time

        t0 = time.time()
        nc = build_nc(int(sys.argv[sys.argv.index("--build") + 1])
                      if len(sys.argv) > 2 else N_ITERS)
        print(f"build ok in {time.time()-t0:.1f}s")
